# revision 19
# baseline (speedup 1.0000x reference)
"""Trainium2 Bass kernel for CrossDecoder kNN-mining margin loss (fp8 edition).

Math: the reference mines, per query q (both columns of train_ill), the k+1
nearest of N=30000 candidates under sum-of-manifolds squared distance
(concat dim 512), then uses those neighbour distances in a margin loss.

Device strategy (SPMD over 8 cores):
  - Rank candidates by score(q,j) = 2q.y_j - |y_j|^2 (descending) computed in
    fp8 (e4m3) DoubleRow matmuls (2 accumulated K=256 matmuls per 256-column
    chunk) -> PSUM fp32.  fp8 adds ~2 abs noise, so the device returns window
    INDICES and the host exact-recomputes the selected windows, making the
    final top-k near-exact (measured rel err ~5e-5).
  - Candidates are globally sorted by |y|^2 and grouped into windows of 32
    with near-constant norm; windows are dealt round-robin to the 8 cores.
    The matmul computes UNBIASED 2q.y; the per-window bias -|y_w|^2 is
    subtracted after windowed max-pooling, so the tensor engine runs at its
    pure-matmul floor (no per-tile bias rewrite).
  - PSUM egress is the bottleneck (only ACT and DVE can read PSUM on TRN2;
    Pool has no compute and cannot touch PSUM; no PSUM access pattern may
    cross a bank boundary or the device hard-faults).  Per 128-query tile
    the 3840 candidates sit in 7 full PSUM banks + 1 half bank: banks 0-1
    are windowed-max-reduced by DVE straight from PSUM, banks 2-7 are
    ACT-copied to bf16 SBUF and max-pooled by a fused 5-level DVE
    pairwise-max tree (tensor_tensor runs 2x on bf16).  Stage 2: DVE
    subtracts the window bias and extracts top-8 values + window indices
    (max8 / max_index) from the 120 window maxima.
Host merges 8 cores x 8 windows x 47 tiles, picks top SELW windows per query
by device value, exact-recomputes those SELW*32 candidate distances in fp64,
and forms the margin loss exactly as the reference does.
"""

import os
import numpy as np

M_, N_, D_, T_ = 2, 30000, 256, 3000
NCORES = 8
KD = M_ * D_                   # 512 contraction dim
QT = 128                       # queries per tile (PSUM partition dim)
# queries are deduplicated (E[distinct of 6000 draws] ~ 5704) and padded to
# 46 tiles; the (astronomically unlikely) overflow recompiles at 47 tiles
NQ_DEFAULT = 5888
QBLK = 4                       # query tiles per output block
CC = 256                       # matmul column chunk (DR moving limit 512)
# per-tile candidate banks: widths and scan policy (d=DVE direct, c=copied)
TBS = (512, 512, 512, 512, 512, 512, 512, 256)
TBPOL = ("d", "c", "c", "c", "c", "c", "c", "c")
NPAD = sum(TBS)                # 3840 candidate slots per core
NDIR = 512                     # leading direct-scanned columns
NCPY = NPAD - NDIR             # 3328 ACT-copied columns
W = 32                         # window width (norm-sorted candidates)
NWIN = NPAD // W               # 120 windows per core
GWIN = (N_ + W - 1) // W       # 938 global windows (last one half real)
SELW = 12                      # windows per query the host exact-recomputes
PAD_BIAS = 30000.0             # pad windows rank last

_cache = {}


def _build_program(NQ):
    import concourse.bass as bass
    import concourse.tile as tile
    from concourse import bacc, mybir

    NQT = NQ // QT
    NBLK = (NQT + QBLK - 1) // QBLK
    dt = mybir.dt
    nc = bacc.Bacc(
        "TRN2", target_bir_lowering=False, debug=False, num_devices=NCORES
    )

    xq_d = nc.dram_tensor("xq", [128, 4, NQ], dt.float8e4, kind="ExternalInput")
    xs_d = nc.dram_tensor("xs", [128, 4, NPAD], dt.float8e4,
                          kind="ExternalInput")
    bias_d = nc.dram_tensor("bias", [128, NWIN], dt.bfloat16,
                            kind="ExternalInput")
    # per query tile: 8 bf16 top values (bitcast as u16) + 8 u16 window ids
    out_d = nc.dram_tensor("out", [NBLK, 128, QBLK * 16], dt.uint16,
                           kind="ExternalOutput")

    DR = mybir.MatmulPerfMode.DoubleRow
    MAX = mybir.AluOpType.max
    TBO = [sum(TBS[:i]) for i in range(len(TBS))]        # column offsets

    with tile.TileContext(nc) as tc:
        with (
            tc.tile_pool(name="res", bufs=1) as res_pool,
            tc.tile_pool(name="cp", bufs=4) as cp_pool,
            tc.tile_pool(name="work", bufs=6) as work_pool,
            tc.tile_pool(name="out", bufs=3) as out_pool,
            tc.tile_pool(name="psum", bufs=8, space=bass.MemorySpace.PSUM) as psum_pool,
        ):
            xs_sb = res_pool.tile([128, 4, NPAD], dt.float8e4, tag="xs")
            for t in range(len(TBS)):
                nc.sync.dma_start(out=xs_sb[:, :, TBO[t]:TBO[t] + TBS[t]],
                                  in_=xs_d[:, :, TBO[t]:TBO[t] + TBS[t]])
            bias_sb = res_pool.tile([128, NWIN], dt.bfloat16, tag="bias")
            nc.sync.dma_start(out=bias_sb[:, :], in_=bias_d[:, :])
            # resident queries, DMA'd in blocks so the first tile starts early
            xq_sb = res_pool.tile([128, 4, NQ], dt.float8e4, tag="xq")
            for blk in range(NBLK):
                q0 = blk * QBLK * QT
                q1 = min(NQ, q0 + QBLK * QT)
                nc.sync.dma_start(out=xq_sb[:, :, q0:q1],
                                  in_=xq_d[:, :, q0:q1])

            for blk in range(NBLK):
                nqt = min(QBLK, NQT - blk * QBLK)
                out_sb = out_pool.tile([128, nqt * 16], dt.uint16, tag="out")
                for jj in range(nqt):
                    j = blk * QBLK + jj
                    win = work_pool.tile([128, NWIN], dt.bfloat16, tag="win")
                    cp = cp_pool.tile([128, NCPY], dt.bfloat16, tag="cp")
                    for tb, (tw, pol) in enumerate(zip(TBS, TBPOL)):
                        # uniform 512-col PSUM tiles (one per bank); the last
                        # 256-col unit just uses half its bank
                        ps = psum_pool.tile([128, 512], dt.float32, tag="ps")
                        for cc in range(tw // CC):
                            for p in range(2):
                                nc.tensor.matmul(
                                    ps[:, cc * CC:(cc + 1) * CC],
                                    lhsT=xq_sb[:, 2 * p:2 * p + 2,
                                               j * QT:(j + 1) * QT],
                                    rhs=xs_sb[:, 2 * p:2 * p + 2,
                                              TBO[tb] + cc * CC:
                                              TBO[tb] + (cc + 1) * CC],
                                    start=(p == 0), stop=(p == 1),
                                    perf_mode=DR,
                                    # 2nd 256-col group shares the PSUM bank;
                                    # the sim's group checker is bank-granular
                                    skip_group_check=(cc == 1),
                                )
                        if pol == "d":
                            ps3 = ps[:, 0:tw].rearrange("p (w j) -> p w j",
                                                        j=W)
                            w0 = TBO[tb] // W
                            nc.vector.tensor_reduce(
                                out=win[:, w0:w0 + tw // W],
                                in_=ps3, axis=mybir.AxisListType.X, op=MAX)
                        else:
                            o = TBO[tb] - NDIR
                            nc.scalar.activation(
                                cp[:, o:o + tw], ps[:, 0:tw],
                                mybir.ActivationFunctionType.Copy)
                    # fused 5-level pairwise-max tree over the copied banks
                    NWC = NCPY // W                          # 88 windows
                    c3 = cp[:, :].rearrange("p (w j) -> p w j", j=W)
                    t1 = work_pool.tile([128, NWC, 16], dt.bfloat16, tag="t1")
                    nc.vector.tensor_tensor(
                        out=t1[:, :, :], in0=c3[:, :, 0:16],
                        in1=c3[:, :, 16:32], op=MAX)
                    t2 = work_pool.tile([128, NWC, 8], dt.bfloat16, tag="t2")
                    nc.vector.tensor_tensor(
                        out=t2[:, :, :], in0=t1[:, :, 0:8],
                        in1=t1[:, :, 8:16], op=MAX)
                    t3 = work_pool.tile([128, NWC, 4], dt.bfloat16, tag="t3")
                    nc.vector.tensor_tensor(
                        out=t3[:, :, :], in0=t2[:, :, 0:4],
                        in1=t2[:, :, 4:8], op=MAX)
                    t4 = work_pool.tile([128, NWC, 2], dt.bfloat16, tag="t4")
                    nc.vector.tensor_tensor(
                        out=t4[:, :, :], in0=t3[:, :, 0:2],
                        in1=t3[:, :, 2:4], op=MAX)
                    nc.vector.tensor_tensor(
                        out=win[:, NDIR // W:], in0=t4[:, :, 0],
                        in1=t4[:, :, 1], op=MAX)
                    biased = work_pool.tile([128, NWIN], dt.bfloat16,
                                            tag="biased")
                    nc.vector.tensor_sub(biased[:, :], win[:, :],
                                         bias_sb[:, :])
                    vals8 = out_sb[:, jj * 16:jj * 16 + 8].bitcast(dt.bfloat16)
                    idx8 = out_sb[:, jj * 16 + 8:jj * 16 + 16]
                    nc.vector.max(vals8, biased[:, :])
                    nc.vector.max_index(idx8, vals8, biased[:, :])
                nc.sync.dma_start(out=out_d[blk, :, :nqt * 16],
                                  in_=out_sb[:, :])

    nc.compile()
    return nc


def _get_program(NQ):
    key = f"nc{NQ}"
    if key not in _cache:
        _cache[key] = _build_program(NQ)
    return _cache[key]


def _prep_inputs(X, q_uniq, NQ):
    """X: [N, 512] fp32; q_uniq: distinct query ids (len <= NQ).

    Returns (per-core input maps, order_norm)."""
    import ml_dtypes
    f8 = ml_dtypes.float8_e4m3

    sq = (X.astype(np.float64) ** 2).sum(1)
    order_norm = np.argsort(sq, kind="stable")          # ascending norm

    nu = len(q_uniq)
    q_idx = np.concatenate([q_uniq, np.zeros(NQ - nu, np.int64)])
    Xq8 = (2.0 * X[q_idx]).astype(f8)
    Xq8[nu:] = 0.0
    # [NQ, 512] -> [128, 4, NQ]: feature kk*128+r at [r, kk, q]
    xq_in = np.ascontiguousarray(
        Xq8.T.reshape(4, 128, NQ).transpose(1, 0, 2))

    Xs8 = np.zeros((GWIN * W, KD), f8)                  # sorted + tail pad
    Xs8[:N_] = X[order_norm].astype(f8)
    sq_sorted = np.full(GWIN * W, np.nan)
    sq_sorted[:N_] = sq[order_norm]
    # per-window bias: mean over REAL members
    sqw = np.nanmean(sq_sorted.reshape(GWIN, W), axis=1)

    per_core = []
    for c in range(NCORES):
        # core c holds global windows g = 8w + c, w = 0..NWIN-1 (g < GWIN)
        gws = 8 * np.arange(NWIN) + c
        valid = gws < GWIN
        xs = np.zeros((NPAD, KD), f8)
        cand_rows = (gws[valid][:, None] * W + np.arange(W)[None, :]).ravel()
        xs[: valid.sum() * W] = Xs8[cand_rows]
        xs_in = np.ascontiguousarray(
            xs.T.reshape(4, 128, NPAD).transpose(1, 0, 2))
        bias = np.full(NWIN, PAD_BIAS, np.float32)
        bias[valid] = sqw[gws[valid]].astype(np.float32)
        bias_in = np.broadcast_to(
            bias.astype(ml_dtypes.bfloat16), (128, NWIN)).copy()
        per_core.append({"xq": xq_in, "xs": xs_in, "bias": bias_in})
    return per_core, order_norm


def _mine(in_maps, NQ, trace=False):
    from concourse.bass_utils import run_bass_kernel_spmd
    import ml_dtypes

    NQT = NQ // QT
    NBLK = (NQT + QBLK - 1) // QBLK
    nc = _get_program(NQ)
    try:
        res = run_bass_kernel_spmd(nc, in_maps, list(range(NCORES)), trace=trace)
    except Exception:
        if not trace:
            raise
        res = run_bass_kernel_spmd(nc, in_maps, list(range(NCORES)), trace=False)
    _cache["last_result"] = res
    vals = np.empty((NQ, NCORES * 8), np.float32)
    wins = np.empty((NQ, NCORES * 8), np.int64)
    for c in range(NCORES):
        o = res.results[c]["out"].reshape(NBLK, 128, QBLK, 16)
        o = o.transpose(0, 2, 1, 3).reshape(NBLK * QBLK * 128, 16)[:NQ]
        vals[:, c * 8:(c + 1) * 8] = (
            o[:, :8].view(ml_dtypes.bfloat16).astype(np.float32))
        # global window id g = 8*slot + core
        wins[:, c * 8:(c + 1) * 8] = 8 * o[:, 8:16].astype(np.int64) + c
    return vals, wins


def kernel(outlayer, c, train_ill, k):
    k = int(k)
    outlayer = np.asarray(outlayer, np.float32)
    train_ill = np.asarray(train_ill)
    X = np.ascontiguousarray(
        outlayer.transpose(1, 0, 2).reshape(N_, KD)).astype(np.float32)
    left = train_ill[:, 0].astype(np.int64)
    right = train_ill[:, 1].astype(np.int64)

    # mine each distinct query once
    q_idx = np.concatenate([right, left])
    q_uniq, q_inv = np.unique(q_idx, return_inverse=True)
    NQ = NQ_DEFAULT
    while len(q_uniq) > NQ:
        NQ += QBLK * QT
    in_maps, order_norm = _prep_inputs(X, q_uniq, NQ)
    vals, wins = _mine(
        in_maps, NQ, trace=bool(int(os.environ.get("KNN_TRACE", "0"))))
    vals = vals[q_inv]                                           # [2t, 64]
    wins = wins[q_inv]

    # host: pick SELW windows per query by device value, exact-recompute
    nq = 2 * T_
    sel = np.argpartition(-vals, SELW - 1, axis=1)[:, :SELW]     # [nq, SELW]
    gsel = np.take_along_axis(wins, sel, axis=1)                 # global wins
    cand_sorted = gsel[:, :, None] * W + np.arange(W)[None, None, :]
    cand_sorted = cand_sorted.reshape(nq, SELW * W)
    pad_mask = cand_sorted >= N_                                 # tail + pad
    cand = order_norm[np.clip(cand_sorted, 0, N_ - 1)]           # original ids

    sq = (X.astype(np.float64) ** 2).sum(1)
    # exact sqdist via |q|^2 + |y|^2 - 2 q.y with per-chunk batched GEMV
    ncand = SELW * W
    B = np.empty((nq, ncand))
    step = 512
    for s in range(0, nq, step):
        e = min(s + step, nq)
        Y = X[cand[s:e]]                                         # [b, nc, 512]
        G = np.einsum("bd,bcd->bc", X[q_idx[s:e]], Y,
                      optimize=True).astype(np.float64)
        B[s:e] = sq[q_idx[s:e], None] + sq[cand[s:e]] - 2.0 * G
    B[pad_mask] = np.inf
    B[cand == q_idx[:, None]] = np.inf                           # drop self
    B = np.sort(B, axis=1)[:, :k]                                # k NN dists

    X64 = X.astype(np.float64)
    D = ((X64[left] - X64[right]) ** 2).sum(1) + 1.0             # [t]
    B2 = B[:T_]                                                  # mining of right
    B1 = B[T_:]                                                  # mining of left
    L1 = np.maximum(D[:, None] - B1, 0.0)
    L2 = np.maximum(D[:, None] - B2, 0.0)
    loss = (L1.mean() + L2.mean()) / 2.0
    return np.asarray(loss, dtype=np.float32)


# revision 20
# speedup vs baseline: 1.1107x; 1.1107x over previous
"""Trainium2 Bass kernel for CrossDecoder kNN-mining margin loss (fp8 edition).

Math: the reference mines, per query q (both columns of train_ill), the k+1
nearest of N=30000 candidates under sum-of-manifolds squared distance
(concat dim 512), then uses those neighbour distances in a margin loss.

Device strategy (SPMD over 8 cores):
  - Rank candidates by score(q,j) = 2q.y_j - |y_j|^2 (descending) computed in
    fp8 (e4m3) DoubleRow matmuls (2 accumulated K=256 matmuls per 256-column
    chunk) -> PSUM fp32.  fp8 adds ~2 abs noise, so the device returns window
    INDICES and the host exact-recomputes the selected windows, making the
    final top-k near-exact (measured rel err ~5e-5).
  - Candidates are globally sorted by |y|^2 and grouped into windows of 32
    with near-constant norm; windows are dealt round-robin to the 8 cores.
    The matmul computes UNBIASED 2q.y; the per-window bias -|y_w|^2 is
    subtracted after windowed max-pooling, so the tensor engine runs at its
    pure-matmul floor (no per-tile bias rewrite).
  - PSUM egress is the bottleneck (only ACT and DVE can read PSUM on TRN2;
    Pool has no compute and cannot touch PSUM; no PSUM access pattern may
    cross a bank boundary or the device hard-faults).  Per 128-query tile
    the 3840 candidates sit in 7 full PSUM banks + 1 half bank: banks 0-1
    are windowed-max-reduced by DVE straight from PSUM, banks 2-7 are
    ACT-copied to bf16 SBUF and max-pooled by a fused 5-level DVE
    pairwise-max tree (tensor_tensor runs 2x on bf16).  Stage 2: DVE
    subtracts the window bias and extracts top-8 values + window indices
    (max8 / max_index) from the 120 window maxima.
Host merges 8 cores x 8 windows x 47 tiles, picks top SELW windows per query
by device value, exact-recomputes those SELW*32 candidate distances in fp64,
and forms the margin loss exactly as the reference does.
"""

import os
import numpy as np

M_, N_, D_, T_ = 2, 30000, 256, 3000
NCORES = 8
KD = M_ * D_                   # 512 contraction dim
QT = 128                       # queries per tile (PSUM partition dim)
# queries are deduplicated (E[distinct of 6000 draws] ~ 5704) and padded to
# 46 tiles; the (astronomically unlikely) overflow recompiles at 47 tiles
NQ_DEFAULT = 5888
QBLK = 4                       # query tiles per output block
CC = 256                       # matmul column chunk (DR moving limit 512)
# per-tile candidate banks: widths and scan policy (d=DVE direct, c=copied)
TBS = (512, 512, 512, 512, 512, 512, 512, 256)
TBPOL = ("d", "d", "c", "c", "c", "c", "c", "c")
NPAD = sum(TBS)                # 3840 candidate slots per core
NDIR = 1024                    # leading direct-scanned columns
NCPY = NPAD - NDIR             # 2816 ACT-copied columns
W = 32                         # window width (norm-sorted candidates)
NWIN = NPAD // W               # 120 windows per core
GWIN = (N_ + W - 1) // W       # 938 global windows (last one half real)
SELW = 16                      # windows per query the host exact-recomputes
PAD_BIAS = 30000.0             # pad windows rank last

_cache = {}


def _build_program(NQ):
    import concourse.bass as bass
    import concourse.tile as tile
    from concourse import bacc, mybir

    NQT = NQ // QT
    NBLK = (NQT + QBLK - 1) // QBLK
    dt = mybir.dt
    nc = bacc.Bacc(
        "TRN2", target_bir_lowering=False, debug=False, num_devices=NCORES
    )

    xq_d = nc.dram_tensor("xq", [128, 4, NQ], dt.float8e4, kind="ExternalInput")
    xs_d = nc.dram_tensor("xs", [128, 4, NPAD], dt.float8e4,
                          kind="ExternalInput")
    bias_d = nc.dram_tensor("bias", [128, NWIN], dt.bfloat16,
                            kind="ExternalInput")
    # per query tile: 8 bf16 top values (bitcast as u16) + 8 u16 window ids
    out_d = nc.dram_tensor("out", [NBLK, 128, QBLK * 16], dt.uint16,
                           kind="ExternalOutput")

    DR = mybir.MatmulPerfMode.DoubleRow
    MAX = mybir.AluOpType.max
    TBO = [sum(TBS[:i]) for i in range(len(TBS))]        # column offsets

    with tile.TileContext(nc) as tc:
        with (
            tc.tile_pool(name="res", bufs=1) as res_pool,
            tc.tile_pool(name="cp", bufs=4) as cp_pool,
            tc.tile_pool(name="work", bufs=6) as work_pool,
            tc.tile_pool(name="out", bufs=3) as out_pool,
            tc.tile_pool(name="psum", bufs=8, space=bass.MemorySpace.PSUM) as psum_pool,
        ):
            xs_sb = res_pool.tile([128, 4, NPAD], dt.float8e4, tag="xs")
            for t in range(len(TBS)):
                nc.sync.dma_start(out=xs_sb[:, :, TBO[t]:TBO[t] + TBS[t]],
                                  in_=xs_d[:, :, TBO[t]:TBO[t] + TBS[t]])
            bias_sb = res_pool.tile([128, NWIN], dt.bfloat16, tag="bias")
            nc.sync.dma_start(out=bias_sb[:, :], in_=bias_d[:, :])
            # resident queries, DMA'd in blocks so the first tile starts early
            xq_sb = res_pool.tile([128, 4, NQ], dt.float8e4, tag="xq")
            for blk in range(NBLK):
                q0 = blk * QBLK * QT
                q1 = min(NQ, q0 + QBLK * QT)
                nc.sync.dma_start(out=xq_sb[:, :, q0:q1],
                                  in_=xq_d[:, :, q0:q1])

            for blk in range(NBLK):
                nqt = min(QBLK, NQT - blk * QBLK)
                out_sb = out_pool.tile([128, nqt * 16], dt.uint16, tag="out")
                for jj in range(nqt):
                    j = blk * QBLK + jj
                    win = work_pool.tile([128, NWIN], dt.bfloat16, tag="win")
                    cp = cp_pool.tile([128, NCPY], dt.bfloat16, tag="cp")
                    for tb, (tw, pol) in enumerate(zip(TBS, TBPOL)):
                        # uniform 512-col PSUM tiles (one per bank); the last
                        # 256-col unit just uses half its bank
                        ps = psum_pool.tile([128, 512], dt.float32, tag="ps")
                        for cc in range(tw // CC):
                            for p in range(2):
                                nc.tensor.matmul(
                                    ps[:, cc * CC:(cc + 1) * CC],
                                    lhsT=xq_sb[:, 2 * p:2 * p + 2,
                                               j * QT:(j + 1) * QT],
                                    rhs=xs_sb[:, 2 * p:2 * p + 2,
                                              TBO[tb] + cc * CC:
                                              TBO[tb] + (cc + 1) * CC],
                                    start=(p == 0), stop=(p == 1),
                                    perf_mode=DR,
                                    # 2nd 256-col group shares the PSUM bank;
                                    # the sim's group checker is bank-granular
                                    skip_group_check=(cc == 1),
                                )
                        if pol == "d":
                            ps3 = ps[:, 0:tw].rearrange("p (w j) -> p w j",
                                                        j=W)
                            w0 = TBO[tb] // W
                            nc.vector.tensor_reduce(
                                out=win[:, w0:w0 + tw // W],
                                in_=ps3, axis=mybir.AxisListType.X, op=MAX)
                        else:
                            o = TBO[tb] - NDIR
                            nc.scalar.activation(
                                cp[:, o:o + tw], ps[:, 0:tw],
                                mybir.ActivationFunctionType.Copy)
                    # fused 5-level pairwise-max tree over the copied banks
                    NWC = NCPY // W                          # 88 windows
                    c3 = cp[:, :].rearrange("p (w j) -> p w j", j=W)
                    t1 = work_pool.tile([128, NWC, 16], dt.bfloat16, tag="t1")
                    nc.vector.tensor_tensor(
                        out=t1[:, :, :], in0=c3[:, :, 0:16],
                        in1=c3[:, :, 16:32], op=MAX)
                    t2 = work_pool.tile([128, NWC, 8], dt.bfloat16, tag="t2")
                    nc.vector.tensor_tensor(
                        out=t2[:, :, :], in0=t1[:, :, 0:8],
                        in1=t1[:, :, 8:16], op=MAX)
                    t3 = work_pool.tile([128, NWC, 4], dt.bfloat16, tag="t3")
                    nc.vector.tensor_tensor(
                        out=t3[:, :, :], in0=t2[:, :, 0:4],
                        in1=t2[:, :, 4:8], op=MAX)
                    t4 = work_pool.tile([128, NWC, 2], dt.bfloat16, tag="t4")
                    nc.vector.tensor_tensor(
                        out=t4[:, :, :], in0=t3[:, :, 0:2],
                        in1=t3[:, :, 2:4], op=MAX)
                    nc.vector.tensor_tensor(
                        out=win[:, NDIR // W:], in0=t4[:, :, 0],
                        in1=t4[:, :, 1], op=MAX)
                    biased = work_pool.tile([128, NWIN], dt.bfloat16,
                                            tag="biased")
                    nc.vector.tensor_sub(biased[:, :], win[:, :],
                                         bias_sb[:, :])
                    vals8 = out_sb[:, jj * 16:jj * 16 + 8].bitcast(dt.bfloat16)
                    idx8 = out_sb[:, jj * 16 + 8:jj * 16 + 16]
                    nc.vector.max(vals8, biased[:, :])
                    nc.vector.max_index(idx8, vals8, biased[:, :])
                nc.sync.dma_start(out=out_d[blk, :, :nqt * 16],
                                  in_=out_sb[:, :])

    nc.compile()
    return nc


def _get_program(NQ):
    key = f"nc{NQ}"
    if key not in _cache:
        _cache[key] = _build_program(NQ)
    return _cache[key]


def _prep_inputs(X, q_uniq, NQ):
    """X: [N, 512] fp32; q_uniq: distinct query ids (len <= NQ).

    Returns (per-core input maps, order_norm)."""
    import ml_dtypes
    f8 = ml_dtypes.float8_e4m3

    sq = (X.astype(np.float64) ** 2).sum(1)
    order_norm = np.argsort(sq, kind="stable")          # ascending norm

    nu = len(q_uniq)
    q_idx = np.concatenate([q_uniq, np.zeros(NQ - nu, np.int64)])
    Xq8 = (2.0 * X[q_idx]).astype(f8)
    Xq8[nu:] = 0.0
    # [NQ, 512] -> [128, 4, NQ]: feature kk*128+r at [r, kk, q]
    xq_in = np.ascontiguousarray(
        Xq8.T.reshape(4, 128, NQ).transpose(1, 0, 2))

    Xs8 = np.zeros((GWIN * W, KD), f8)                  # sorted + tail pad
    Xs8[:N_] = X[order_norm].astype(f8)
    sq_sorted = np.full(GWIN * W, np.nan)
    sq_sorted[:N_] = sq[order_norm]
    # per-window bias: mean over REAL members
    sqw = np.nanmean(sq_sorted.reshape(GWIN, W), axis=1)

    per_core = []
    for c in range(NCORES):
        # core c holds global windows g = 8w + c, w = 0..NWIN-1 (g < GWIN)
        gws = 8 * np.arange(NWIN) + c
        valid = gws < GWIN
        xs = np.zeros((NPAD, KD), f8)
        cand_rows = (gws[valid][:, None] * W + np.arange(W)[None, :]).ravel()
        xs[: valid.sum() * W] = Xs8[cand_rows]
        xs_in = np.ascontiguousarray(
            xs.T.reshape(4, 128, NPAD).transpose(1, 0, 2))
        bias = np.full(NWIN, PAD_BIAS, np.float32)
        bias[valid] = sqw[gws[valid]].astype(np.float32)
        bias_in = np.broadcast_to(
            bias.astype(ml_dtypes.bfloat16), (128, NWIN)).copy()
        per_core.append({"xq": xq_in, "xs": xs_in, "bias": bias_in})
    return per_core, order_norm


def _mine(in_maps, NQ, trace=False):
    from concourse.bass_utils import run_bass_kernel_spmd
    import ml_dtypes

    NQT = NQ // QT
    NBLK = (NQT + QBLK - 1) // QBLK
    nc = _get_program(NQ)
    try:
        res = run_bass_kernel_spmd(nc, in_maps, list(range(NCORES)), trace=trace)
    except Exception:
        if not trace:
            raise
        res = run_bass_kernel_spmd(nc, in_maps, list(range(NCORES)), trace=False)
    _cache["last_result"] = res
    vals = np.empty((NQ, NCORES * 8), np.float32)
    wins = np.empty((NQ, NCORES * 8), np.int64)
    for c in range(NCORES):
        o = res.results[c]["out"].reshape(NBLK, 128, QBLK, 16)
        o = o.transpose(0, 2, 1, 3).reshape(NBLK * QBLK * 128, 16)[:NQ]
        vals[:, c * 8:(c + 1) * 8] = (
            o[:, :8].view(ml_dtypes.bfloat16).astype(np.float32))
        # global window id g = 8*slot + core
        wins[:, c * 8:(c + 1) * 8] = 8 * o[:, 8:16].astype(np.int64) + c
    return vals, wins


def kernel(outlayer, c, train_ill, k):
    k = int(k)
    outlayer = np.asarray(outlayer, np.float32)
    train_ill = np.asarray(train_ill)
    X = np.ascontiguousarray(
        outlayer.transpose(1, 0, 2).reshape(N_, KD)).astype(np.float32)
    left = train_ill[:, 0].astype(np.int64)
    right = train_ill[:, 1].astype(np.int64)

    # mine each distinct query once
    q_idx = np.concatenate([right, left])
    q_uniq, q_inv = np.unique(q_idx, return_inverse=True)
    NQ = NQ_DEFAULT
    while len(q_uniq) > NQ:
        NQ += QBLK * QT
    in_maps, order_norm = _prep_inputs(X, q_uniq, NQ)
    vals, wins = _mine(
        in_maps, NQ, trace=bool(int(os.environ.get("KNN_TRACE", "0"))))
    vals = vals[q_inv]                                           # [2t, 64]
    wins = wins[q_inv]

    # host: pick SELW windows per query by device value, exact-recompute
    nq = 2 * T_
    sel = np.argpartition(-vals, SELW - 1, axis=1)[:, :SELW]     # [nq, SELW]
    gsel = np.take_along_axis(wins, sel, axis=1)                 # global wins
    cand_sorted = gsel[:, :, None] * W + np.arange(W)[None, None, :]
    cand_sorted = cand_sorted.reshape(nq, SELW * W)
    pad_mask = cand_sorted >= N_                                 # tail + pad
    cand = order_norm[np.clip(cand_sorted, 0, N_ - 1)]           # original ids

    sq = (X.astype(np.float64) ** 2).sum(1)
    # exact sqdist via |q|^2 + |y|^2 - 2 q.y with per-chunk batched GEMV
    ncand = SELW * W
    B = np.empty((nq, ncand))
    step = 512
    for s in range(0, nq, step):
        e = min(s + step, nq)
        Y = X[cand[s:e]]                                         # [b, nc, 512]
        G = np.einsum("bd,bcd->bc", X[q_idx[s:e]], Y,
                      optimize=True).astype(np.float64)
        B[s:e] = sq[q_idx[s:e], None] + sq[cand[s:e]] - 2.0 * G
    B[pad_mask] = np.inf
    B[cand == q_idx[:, None]] = np.inf                           # drop self
    B = np.sort(B, axis=1)[:, :k]                                # k NN dists

    X64 = X.astype(np.float64)
    D = ((X64[left] - X64[right]) ** 2).sum(1) + 1.0             # [t]
    B2 = B[:T_]                                                  # mining of right
    B1 = B[T_:]                                                  # mining of left
    L1 = np.maximum(D[:, None] - B1, 0.0)
    L2 = np.maximum(D[:, None] - B2, 0.0)
    loss = (L1.mean() + L2.mean()) / 2.0
    return np.asarray(loss, dtype=np.float32)


# revision 23
# speedup vs baseline: 1.1238x; 1.0118x over previous
"""Trainium2 Bass kernel for CrossDecoder kNN-mining margin loss (fp8 edition).

Math: the reference mines, per query q (both columns of train_ill), the k+1
nearest of N=30000 candidates under sum-of-manifolds squared distance
(concat dim 512), then uses those neighbour distances in a margin loss.

Device strategy (SPMD over 8 cores):
  - Rank candidates by score(q,j) = 2q.y_j - |y_j|^2 (descending) computed in
    fp8 (e4m3) DoubleRow matmuls (2 accumulated K=256 matmuls per 256-column
    chunk) -> PSUM fp32.  fp8 adds ~2 abs noise, so the device returns window
    INDICES and the host exact-recomputes the selected windows, making the
    final top-k near-exact (measured rel err ~5e-5).
  - Candidates are globally sorted by |y|^2 and grouped into windows of 32
    with near-constant norm; windows are dealt round-robin to the 8 cores.
    The matmul computes UNBIASED 2q.y; the per-window bias -|y_w|^2 is
    subtracted after windowed max-pooling, so the tensor engine runs at its
    pure-matmul floor (no per-tile bias rewrite).
  - PSUM egress is the bottleneck (only ACT and DVE can read PSUM on TRN2;
    Pool has no compute and cannot touch PSUM; no PSUM access pattern may
    cross a bank boundary or the device hard-faults).  Per 128-query tile
    the 3840 candidates sit in 7 full PSUM banks + 1 half bank: banks 0-1
    are windowed-max-reduced by DVE straight from PSUM, banks 2-7 are
    ACT-copied to bf16 SBUF and max-pooled by a fused 5-level DVE
    pairwise-max tree (tensor_tensor runs 2x on bf16).  Stage 2: DVE
    subtracts the window bias and extracts top-8 values + window indices
    (max8 / max_index) from the 120 window maxima.
Host merges 8 cores x 8 windows x 47 tiles, picks top SELW windows per query
by device value, exact-recomputes those SELW*32 candidate distances in fp64,
and forms the margin loss exactly as the reference does.
"""

import os
import numpy as np

M_, N_, D_, T_ = 2, 30000, 256, 3000
NCORES = 8
KD = M_ * D_                   # 512 contraction dim
QT = 128                       # queries per tile (PSUM partition dim)
# queries are deduplicated (E[distinct of 6000 draws] ~ 5704) and padded to
# 46 tiles; the (astronomically unlikely) overflow recompiles at 47 tiles
NQ_DEFAULT = 5888
QBLK = 4                       # query tiles per output block
CC = 256                       # matmul column chunk (DR moving limit 512)
# per-tile candidate banks: widths and scan policy (d=DVE direct, c=copied)
TBS = (512, 512, 512, 512, 512, 512, 512, 256)
TBPOL = ("d", "d", "c", "c", "c", "c", "c", "c")
NPAD = sum(TBS)                # 3840 candidate slots per core
NDIR = 1024                    # leading direct-scanned columns
NCPY = NPAD - NDIR             # 2816 ACT-copied columns
W = 32                         # window width (norm-sorted candidates)
NWIN = NPAD // W               # 120 windows per core
GWIN = (N_ + W - 1) // W       # 938 global windows (last one half real)
SELW = 16                      # windows per query the host exact-recomputes
PAD_BIAS = 30000.0             # pad windows rank last

_cache = {}


def _build_program(NQ):
    import concourse.bass as bass
    import concourse.tile as tile
    from concourse import bacc, mybir

    NQT = NQ // QT
    NBLK = (NQT + QBLK - 1) // QBLK
    dt = mybir.dt
    nc = bacc.Bacc(
        "TRN2", target_bir_lowering=False, debug=False, num_devices=NCORES
    )

    xq_d = nc.dram_tensor("xq", [128, 4, NQ], dt.float8e4, kind="ExternalInput")
    xs_d = nc.dram_tensor("xs", [128, 4, NPAD], dt.float8e4,
                          kind="ExternalInput")
    bias_d = nc.dram_tensor("bias", [128, NWIN], dt.bfloat16,
                            kind="ExternalInput")
    # per query tile: 8 bf16 top values (bitcast as u16) + 8 u16 window ids
    out_d = nc.dram_tensor("out", [NBLK, 128, QBLK * 16], dt.uint16,
                           kind="ExternalOutput")

    DR = mybir.MatmulPerfMode.DoubleRow
    MAX = mybir.AluOpType.max
    TBO = [sum(TBS[:i]) for i in range(len(TBS))]        # column offsets

    with tile.TileContext(nc) as tc:
        with (
            tc.tile_pool(name="res", bufs=1) as res_pool,
            tc.tile_pool(name="cp", bufs=4) as cp_pool,
            tc.tile_pool(name="work", bufs=6) as work_pool,
            tc.tile_pool(name="out", bufs=3) as out_pool,
            tc.tile_pool(name="psum", bufs=8, space=bass.MemorySpace.PSUM) as psum_pool,
        ):
            xs_sb = res_pool.tile([128, 4, NPAD], dt.float8e4, tag="xs")
            for t in range(len(TBS)):
                nc.sync.dma_start(out=xs_sb[:, :, TBO[t]:TBO[t] + TBS[t]],
                                  in_=xs_d[:, :, TBO[t]:TBO[t] + TBS[t]])
            bias_sb = res_pool.tile([128, NWIN], dt.bfloat16, tag="bias")
            nc.sync.dma_start(out=bias_sb[:, :], in_=bias_d[:, :])
            # resident queries, DMA'd in blocks so the first tile starts early
            xq_sb = res_pool.tile([128, 4, NQ], dt.float8e4, tag="xq")
            for blk in range(NBLK):
                q0 = blk * QBLK * QT
                q1 = min(NQ, q0 + QBLK * QT)
                nc.sync.dma_start(out=xq_sb[:, :, q0:q1],
                                  in_=xq_d[:, :, q0:q1])

            # PSUM layout per query tile: fused two-bank tiles where the
            # consumer reads both banks in one instruction.  Units:
            # (width, policy): d = DVE windowed reduce, c = ACT copy.
            UNITS = ((1024, "d"), (1024, "c"), (1024, "c"), (512, "c"),
                     (256, "c"))
            state = {}

            def emit_front(j):
                """matmuls + PSUM egress (ACT copies / DVE reduces)."""
                win = work_pool.tile([128, NWIN], dt.bfloat16, tag="win",
                                     name="win")
                cp = cp_pool.tile([128, NCPY], dt.bfloat16, tag="cp",
                                  name="cp")
                off = 0
                for ui, (tw, pol) in enumerate(UNITS):
                    ps = psum_pools[ui].tile([128, max(tw, 512)], dt.float32,
                                             tag=f"ps{ui}", name=f"ps{ui}")
                    for cc in range(tw // CC):
                        for p in range(2):
                            nc.tensor.matmul(
                                ps[:, cc * CC:(cc + 1) * CC],
                                lhsT=xq_sb[:, 2 * p:2 * p + 2,
                                           j * QT:(j + 1) * QT],
                                rhs=xs_sb[:, 2 * p:2 * p + 2,
                                          off + cc * CC:off + (cc + 1) * CC],
                                start=(p == 0), stop=(p == 1),
                                perf_mode=DR,
                                # every 2nd 256-col group shares a PSUM bank;
                                # the sim's group checker is bank-granular
                                skip_group_check=(cc % 2 == 1),
                            )
                    if pol == "d":
                        ps3 = ps[:, 0:tw].rearrange("p (w j) -> p w j", j=W)
                        nc.vector.tensor_reduce(
                            out=win[:, off // W:(off + tw) // W],
                            in_=ps3, axis=mybir.AxisListType.X, op=MAX)
                    else:
                        o = off - NDIR
                        nc.scalar.activation(
                            cp[:, o:o + tw], ps[:, 0:tw],
                            mybir.ActivationFunctionType.Copy)
                    off += tw
                state[j] = (win, cp)

            def emit_back(j):
                """SBUF-side max tree + top-8 stage for tile j (deferred one
                iteration so DVE never head-of-line blocks on ACT copies)."""
                win, cp = state.pop(j)
                blk, jj = divmod(j, QBLK)
                nqt = min(QBLK, NQT - blk * QBLK)
                if jj == 0:
                    state[("out", blk)] = out_pool.tile(
                        [128, nqt * 16], dt.uint16, tag="out", name="out_sb")
                out_sb = state[("out", blk)]
                NWC = NCPY // W
                c3 = cp[:, :].rearrange("p (w j) -> p w j", j=W)
                t1 = work_pool.tile([128, NWC, 16], dt.bfloat16, tag="t1")
                nc.vector.tensor_tensor(
                    out=t1[:, :, :], in0=c3[:, :, 0:16],
                    in1=c3[:, :, 16:32], op=MAX)
                t2 = work_pool.tile([128, NWC, 8], dt.bfloat16, tag="t2")
                nc.vector.tensor_tensor(
                    out=t2[:, :, :], in0=t1[:, :, 0:8],
                    in1=t1[:, :, 8:16], op=MAX)
                t3 = work_pool.tile([128, NWC, 4], dt.bfloat16, tag="t3")
                nc.vector.tensor_tensor(
                    out=t3[:, :, :], in0=t2[:, :, 0:4],
                    in1=t2[:, :, 4:8], op=MAX)
                t4 = work_pool.tile([128, NWC, 2], dt.bfloat16, tag="t4")
                nc.vector.tensor_tensor(
                    out=t4[:, :, :], in0=t3[:, :, 0:2],
                    in1=t3[:, :, 2:4], op=MAX)
                nc.vector.tensor_tensor(
                    out=win[:, NDIR // W:], in0=t4[:, :, 0],
                    in1=t4[:, :, 1], op=MAX)
                biased = work_pool.tile([128, NWIN], dt.bfloat16,
                                        tag="biased")
                nc.vector.tensor_sub(biased[:, :], win[:, :], bias_sb[:, :])
                vals8 = out_sb[:, jj * 16:jj * 16 + 8].bitcast(dt.bfloat16)
                idx8 = out_sb[:, jj * 16 + 8:jj * 16 + 16]
                nc.vector.max(vals8, biased[:, :])
                nc.vector.max_index(idx8, vals8, biased[:, :])
                if jj == nqt - 1:
                    nc.sync.dma_start(out=out_d[blk, :, :nqt * 16],
                                      in_=out_sb[:, :])
                    del state[("out", blk)]

            from contextlib import ExitStack
            with ExitStack() as psum_stack:
                psum_pools = [
                    psum_stack.enter_context(tc.tile_pool(
                        name=f"psum{ui}", bufs=1,
                        space=bass.MemorySpace.PSUM))
                    for ui in range(len(UNITS))
                ]
                for j in range(NQT):
                    emit_front(j)
                    if j > 0:
                        emit_back(j - 1)
                emit_back(NQT - 1)

    nc.compile()
    return nc


def _get_program(NQ):
    key = f"nc{NQ}"
    if key not in _cache:
        _cache[key] = _build_program(NQ)
    return _cache[key]


def _prep_inputs(X, q_uniq, NQ):
    """X: [N, 512] fp32; q_uniq: distinct query ids (len <= NQ).

    Returns (per-core input maps, order_norm)."""
    import ml_dtypes
    f8 = ml_dtypes.float8_e4m3

    sq = (X.astype(np.float64) ** 2).sum(1)
    order_norm = np.argsort(sq, kind="stable")          # ascending norm

    nu = len(q_uniq)
    q_idx = np.concatenate([q_uniq, np.zeros(NQ - nu, np.int64)])
    Xq8 = (2.0 * X[q_idx]).astype(f8)
    Xq8[nu:] = 0.0
    # [NQ, 512] -> [128, 4, NQ]: feature kk*128+r at [r, kk, q]
    xq_in = np.ascontiguousarray(
        Xq8.T.reshape(4, 128, NQ).transpose(1, 0, 2))

    Xs8 = np.zeros((GWIN * W, KD), f8)                  # sorted + tail pad
    Xs8[:N_] = X[order_norm].astype(f8)
    sq_sorted = np.full(GWIN * W, np.nan)
    sq_sorted[:N_] = sq[order_norm]
    # per-window bias: mean over REAL members
    sqw = np.nanmean(sq_sorted.reshape(GWIN, W), axis=1)

    per_core = []
    for c in range(NCORES):
        # core c holds global windows g = 8w + c, w = 0..NWIN-1 (g < GWIN)
        gws = 8 * np.arange(NWIN) + c
        valid = gws < GWIN
        xs = np.zeros((NPAD, KD), f8)
        cand_rows = (gws[valid][:, None] * W + np.arange(W)[None, :]).ravel()
        xs[: valid.sum() * W] = Xs8[cand_rows]
        xs_in = np.ascontiguousarray(
            xs.T.reshape(4, 128, NPAD).transpose(1, 0, 2))
        bias = np.full(NWIN, PAD_BIAS, np.float32)
        bias[valid] = sqw[gws[valid]].astype(np.float32)
        bias_in = np.broadcast_to(
            bias.astype(ml_dtypes.bfloat16), (128, NWIN)).copy()
        per_core.append({"xq": xq_in, "xs": xs_in, "bias": bias_in})
    return per_core, order_norm


def _mine(in_maps, NQ, trace=False):
    from concourse.bass_utils import run_bass_kernel_spmd
    import ml_dtypes

    NQT = NQ // QT
    NBLK = (NQT + QBLK - 1) // QBLK
    nc = _get_program(NQ)
    try:
        res = run_bass_kernel_spmd(nc, in_maps, list(range(NCORES)), trace=trace)
    except Exception:
        if not trace:
            raise
        res = run_bass_kernel_spmd(nc, in_maps, list(range(NCORES)), trace=False)
    _cache["last_result"] = res
    vals = np.empty((NQ, NCORES * 8), np.float32)
    wins = np.empty((NQ, NCORES * 8), np.int64)
    for c in range(NCORES):
        o = res.results[c]["out"].reshape(NBLK, 128, QBLK, 16)
        o = o.transpose(0, 2, 1, 3).reshape(NBLK * QBLK * 128, 16)[:NQ]
        vals[:, c * 8:(c + 1) * 8] = (
            o[:, :8].view(ml_dtypes.bfloat16).astype(np.float32))
        # global window id g = 8*slot + core
        wins[:, c * 8:(c + 1) * 8] = 8 * o[:, 8:16].astype(np.int64) + c
    return vals, wins


def kernel(outlayer, c, train_ill, k):
    k = int(k)
    outlayer = np.asarray(outlayer, np.float32)
    train_ill = np.asarray(train_ill)
    X = np.ascontiguousarray(
        outlayer.transpose(1, 0, 2).reshape(N_, KD)).astype(np.float32)
    left = train_ill[:, 0].astype(np.int64)
    right = train_ill[:, 1].astype(np.int64)

    # mine each distinct query once
    q_idx = np.concatenate([right, left])
    q_uniq, q_inv = np.unique(q_idx, return_inverse=True)
    NQ = NQ_DEFAULT
    while len(q_uniq) > NQ:
        NQ += QBLK * QT
    in_maps, order_norm = _prep_inputs(X, q_uniq, NQ)
    vals, wins = _mine(
        in_maps, NQ, trace=bool(int(os.environ.get("KNN_TRACE", "0"))))
    vals = vals[q_inv]                                           # [2t, 64]
    wins = wins[q_inv]

    # host: pick SELW windows per query by device value, exact-recompute
    nq = 2 * T_
    sel = np.argpartition(-vals, SELW - 1, axis=1)[:, :SELW]     # [nq, SELW]
    gsel = np.take_along_axis(wins, sel, axis=1)                 # global wins
    cand_sorted = gsel[:, :, None] * W + np.arange(W)[None, None, :]
    cand_sorted = cand_sorted.reshape(nq, SELW * W)
    pad_mask = cand_sorted >= N_                                 # tail + pad
    cand = order_norm[np.clip(cand_sorted, 0, N_ - 1)]           # original ids

    sq = (X.astype(np.float64) ** 2).sum(1)
    # exact sqdist via |q|^2 + |y|^2 - 2 q.y with per-chunk batched GEMV
    ncand = SELW * W
    B = np.empty((nq, ncand))
    step = 512
    for s in range(0, nq, step):
        e = min(s + step, nq)
        Y = X[cand[s:e]]                                         # [b, nc, 512]
        G = np.einsum("bd,bcd->bc", X[q_idx[s:e]], Y,
                      optimize=True).astype(np.float64)
        B[s:e] = sq[q_idx[s:e], None] + sq[cand[s:e]] - 2.0 * G
    B[pad_mask] = np.inf
    B[cand == q_idx[:, None]] = np.inf                           # drop self
    B = np.sort(B, axis=1)[:, :k]                                # k NN dists

    X64 = X.astype(np.float64)
    D = ((X64[left] - X64[right]) ** 2).sum(1) + 1.0             # [t]
    B2 = B[:T_]                                                  # mining of right
    B1 = B[T_:]                                                  # mining of left
    L1 = np.maximum(D[:, None] - B1, 0.0)
    L2 = np.maximum(D[:, None] - B2, 0.0)
    loss = (L1.mean() + L2.mean()) / 2.0
    return np.asarray(loss, dtype=np.float32)
